# revision 25
# baseline (speedup 1.0000x reference)
"""DeformGAT (4-layer) Trainium2 kernel — 8 NeuronCores SPMD.

Sharding: nodes in 8 contiguous blocks of 1250 (padded to 1280); edges are
assigned to their dst node's core (edges pre-sorted by dst on host). Weights
replicated. Feature tables are shipped as fp8 rows with a bf16 sidecar
([din fp8 | 6 scores bf16 | 2 coords bf16 | pad]), roughly halving AllGather
and gather traffic; fp8 is only used on the attention/feature W-path (final
rel err ~1e-3, validated against the fp32 reference in numpy). Per layer each
core gathers src rows of the replicated table (dma_gather), computes the
per-edge segment softmax with three slab-wide one-hot matmuls (dst-score
broadcast, denominator reduce, reciprocal broadcast) plus vector masking,
aggregates features with fp8 scatter matmuls, aggregates the attention-
weighted src coordinates with a fourth one-hot matmul (bf16 sidecar), then
ships its produced rows via fine-grained chunked AllGathers that overlap the
slab loop.
"""
import numpy as np
import ml_dtypes
from contextlib import ExitStack

import concourse.bacc as bacc
import concourse.bass as bass
import concourse.tile as tile
import concourse.mybir as mybir
from concourse import library_config
from concourse.bass_utils import run_bass_kernel_spmd

F32 = mybir.dt.float32
BF16 = mybir.dt.bfloat16
F8 = mybir.dt.float8e4
I16 = mybir.dt.int16
AF = mybir.ActivationFunctionType
BF = ml_dtypes.bfloat16

NCORES = 8
N = 10000
E = 60000
H = 6
NL = 1250          # real nodes per core
NLP = 1280         # padded nodes per core
NBLK = 80          # dst blocks of 16 per core
NSLAB = 10         # slabs of 128 dst nodes (8 blocks)
CAP = 128          # edge capacity per block (= chunk)
NCH = 5            # AllGather chunks per stage (2 slabs each)
CHROWS = 256       # rows per AllGather chunk

# GAT layer dims (din, C). Stage s (2..5) runs GAT s-1.
GAT_DIMS = [(256, 508), (512, 250), (256, 120), (128, 20)]
FDIM = [256, 512, 256, 128]          # feat_s dim produced by stage s
FP8_ROWS = True
# ROWB: table row width in bf16 elements. All DRAM transport (agin, feat,
# AllGather, dma_gather) is bf16-typed: shipping fp8-typed tensors corrupts
# bf16 sidecar bytes whose low byte matches an fp8 NaN pattern (canonicalized
# to 0x7f in the datapath). Valid e4m3 feature bytes are <= 0xf7, so fp8
# byte-pairs can never alias a bf16 NaN/Inf — bf16 transport is bit-safe.
if FP8_ROWS:
    ROWB = [256, 384, 256, 128]      # din/2 fp8-byte-pairs + 8 sidecar elems
else:
    ROWB = [384, 640, 384, 256]      # din + 8 sidecar elems

SELU_L = 1.0507009873554805
SELU_A = 1.6732632423543772


def _build_nc():
    nc = bacc.Bacc("TRN2", target_bir_lowering=False, debug=False,
                   num_devices=NCORES)
    # ---------------- inputs ----------------
    inp = {}
    inp["dataT"] = nc.dram_tensor("dataT", [16, NLP], F32, kind="ExternalInput")
    inp["coords_loc"] = nc.dram_tensor("coords_loc", [NLP, 2], F32, kind="ExternalInput")
    inp["cfac"] = nc.dram_tensor("cfac", [NLP, 1], F32, kind="ExternalInput")
    inp["srcidx"] = nc.dram_tensor("srcidx", [128, NBLK * 8], I16, kind="ExternalInput")
    inp["Sbc"] = nc.dram_tensor("Sbc", [128, NSLAB * 128], BF16, kind="ExternalInput")
    inp["Str"] = nc.dram_tensor("Str", [128, NSLAB * 128], BF16, kind="ExternalInput")
    inp["maskrep"] = nc.dram_tensor("maskrep", [128, 8 * 6], F32, kind="ExternalInput")
    inp["maskd"] = nc.dram_tensor("maskd", [128, 8 * 2], F32, kind="ExternalInput")
    inp["p0rep"] = nc.dram_tensor("p0rep", [128, NBLK * 96], BF16, kind="ExternalInput")
    inp["ident"] = nc.dram_tensor("ident", [128, 128], BF16, kind="ExternalInput")
    inp["linW"] = nc.dram_tensor("linW", [16, 254], F32, kind="ExternalInput")
    inp["bias1"] = nc.dram_tensor("bias1", [128, 254], F32, kind="ExternalInput")
    for i in range(1, 5):
        din, C = GAT_DIMS[i - 1]
        kt = din // 128
        if i < 4:
            inp[f"wp{i}"] = nc.dram_tensor(f"wp{i}", [128, kt * 6 * C], BF16, kind="ExternalInput")
            inp[f"biasg{i}"] = nc.dram_tensor(f"biasg{i}", [128, C], F32, kind="ExternalInput")
        ktf = FDIM[i - 1] // 128
        inp[f"wsc{i}"] = nc.dram_tensor(f"wsc{i}", [128, ktf * 12], BF16, kind="ExternalInput")
    out_t = nc.dram_tensor("out", [NLP, 2], F32, kind="ExternalOutput")
    DBG = False
    if DBG:
        dbg_fg = nc.dram_tensor("dbg_fg", [128, 8 * ROWB[0]], F32, kind="ExternalOutput")
        dbg_ex = nc.dram_tensor("dbg_ex", [128, 48], F32, kind="ExternalOutput")
        dbg_al = nc.dram_tensor("dbg_al", [128, 48], F32, kind="ExternalOutput")
        dbg_pf = nc.dram_tensor("dbg_pf", [128, 508], F32, kind="ExternalOutput")
        dbg_da = nc.dram_tensor("dbg_da", [128, 2], F32, kind="ExternalOutput")
        dbg_fn = nc.dram_tensor("dbg_fn", [128, 512], F32, kind="ExternalOutput")
        dbg_fn3 = nc.dram_tensor("dbg_fn3", [128, 256], F32, kind="ExternalOutput")
        dbg_fn4 = nc.dram_tensor("dbg_fn4", [128, 128], F32, kind="ExternalOutput")

    rg = [list(range(NCORES))]

    with tile.TileContext(nc) as tc, ExitStack() as ctx:
        persist = ctx.enter_context(tc.tile_pool(name="persist", bufs=1))
        dram = ctx.enter_context(tc.tile_pool(name="dram", bufs=1, space="DRAM"))
        fg_pool = ctx.enter_context(tc.tile_pool(name="fg", bufs=4))
        gt_pool = ctx.enter_context(tc.tile_pool(name="gt", bufs=2))
        m_pool = ctx.enter_context(tc.tile_pool(name="m", bufs=2))
        e_pool = ctx.enter_context(tc.tile_pool(name="ep", bufs=3))
        fn_pool = ctx.enter_context(tc.tile_pool(name="fn", bufs=2))
        fn8_pool = ctx.enter_context(tc.tile_pool(name="fn8", bufs=2))
        fnt_pool = ctx.enter_context(tc.tile_pool(name="fnt", bufs=2))
        wp_pool = ctx.enter_context(tc.tile_pool(name="wp", bufs=2))
        small = ctx.enter_context(tc.tile_pool(name="small", bufs=3))
        ps_gt = ctx.enter_context(tc.tile_pool(name="psgt", bufs=2, space="PSUM"))
        ps_f = ctx.enter_context(tc.tile_pool(name="psf", bufs=2, space="PSUM"))
        ps_sm = ctx.enter_context(tc.tile_pool(name="pssm", bufs=2, space="PSUM"))

        nc.gpsimd.load_library(library_config.mlp)

        # ------------- resident loads -------------
        srcidx_sb = persist.tile([128, NBLK * 8], I16)
        nc.sync.dma_start(srcidx_sb[:], inp["srcidx"][:])
        S_sb = persist.tile([128, NSLAB * 128], BF16)
        nc.sync.dma_start(S_sb[:], inp["Sbc"][:])
        St_sb = persist.tile([128, NSLAB * 128], BF16)
        nc.sync.dma_start(St_sb[:], inp["Str"][:])
        mask_sb = persist.tile([128, 8, 6], F32)
        nc.sync.dma_start(mask_sb[:], inp["maskrep"][:].rearrange("p (b h) -> p b h", b=8))
        maskd_sb = persist.tile([128, 8, 2], F32)
        nc.sync.dma_start(maskd_sb[:], inp["maskd"][:].rearrange("p (b c) -> p b c", b=8))
        p0rep_sb = persist.tile([128, NBLK * 96], BF16)
        nc.sync.dma_start(p0rep_sb[:], inp["p0rep"][:])
        ident_sb = persist.tile([128, 128], BF16)
        nc.sync.dma_start(ident_sb[:], inp["ident"][:])
        dataT_sb = persist.tile([16, NLP], F32)
        nc.sync.dma_start(dataT_sb[:], inp["dataT"][:])
        linW_sb = persist.tile([16, 254], F32)
        nc.sync.dma_start(linW_sb[:], inp["linW"][:])
        bias1_sb = persist.tile([128, 254], F32)
        nc.sync.dma_start(bias1_sb[:], inp["bias1"][:])
        cloc_sb = persist.tile([128, NSLAB, 2], F32)
        nc.sync.dma_start(cloc_sb[:],
                          inp["coords_loc"][:].rearrange("(s p) c -> p s c", p=128))
        cfac_sb = persist.tile([128, NSLAB, 1], F32)
        nc.sync.dma_start(cfac_sb[:],
                          inp["cfac"][:].rearrange("(s p) c -> p s c", p=128))
        wsc_sb = {}
        for i in range(1, 5):
            ktf = FDIM[i - 1] // 128
            t = persist.tile([128, ktf * 12], BF16, tag=f"wsc{i}", name=f"wsc{i}_sb")
            nc.sync.dma_start(t[:], inp[f"wsc{i}"][:])
            wsc_sb[i] = t
        biasg_sb = {}
        for i in range(1, 4):
            C = GAT_DIMS[i - 1][1]
            t = persist.tile([128, C], F32, tag=f"biasg{i}", name=f"biasg{i}_sb")
            nc.sync.dma_start(t[:], inp[f"biasg{i}"][:])
            biasg_sb[i] = t

        # per-stage state
        SDall = persist.tile([128, NSLAB, 6], F32)
        CSTK = persist.tile([128, NSLAB, 8], F32)
        OUTC = persist.tile([128, NSLAB, 2], F32)

        # DRAM tables (fp8 rows). agin is split per AllGather chunk so each
        # collective depends only on the two slab writes that feed it.
        agin = {}
        feat = {}
        for s in range(1, 5):
            agin[s] = [dram.tile([CHROWS, ROWB[s - 1]], BF16,
                                 tag=f"agin{s}_{k}", name=f"agin{s}_{k}")
                       for k in range(NCH)]
            feat[s] = dram.tile([NCORES * NLP, ROWB[s - 1]], BF16, tag=f"feat{s}",
                                name=f"feat{s}")

        # =========================================================
        def selu_into(dst_ap, psum_ap, bias_ap, C):
            """dst = selu(psum[:, :C] + bias)  (dst may be bf16)"""
            t1 = e_pool.tile([128, C], F32, tag="selu_t1")
            nc.vector.tensor_add(t1[:], psum_ap, bias_ap)
            mn = e_pool.tile([128, C], F32, tag="selu_mn")
            nc.scalar.activation(mn[:], t1[:], AF.Relu, scale=-1.0)
            ex = e_pool.tile([128, C], F32, tag="selu_ex")
            nc.scalar.activation(ex[:], mn[:], AF.Exp, scale=-1.0)
            em = e_pool.tile([128, C], F32, tag="selu_em")
            nc.vector.tensor_scalar(em[:], ex[:], SELU_L * SELU_A,
                                    -SELU_L * SELU_A,
                                    mybir.AluOpType.mult, mybir.AluOpType.add)
            rp = e_pool.tile([128, C], F32, tag="selu_rp")
            nc.scalar.activation(rp[:], t1[:], AF.Relu, scale=SELU_L)
            nc.vector.tensor_add(dst_ap, em[:], rp[:])

        def produce(stage, s, psum_f, dispagg):
            """assemble the row for GAT layer `stage` of slab s; scores; ship.

            FNb (bf16 [128, din]) feeds the score transposes; FN8 is the
            shipped fp8 row with the bf16 sidecar."""
            din_out = FDIM[stage - 1]
            rb = ROWB[stage - 1]
            FNb = fn_pool.tile([128, din_out], BF16, tag="FNb")
            FN8 = fn8_pool.tile([128, 2 * rb] if FP8_ROWS else [128, rb],
                                F8 if FP8_ROWS else BF16, tag="FN8")
            if stage == 1:
                nc.vector.tensor_copy(FNb[:, 0:2], cloc_sb[:, s, :])
                nc.vector.tensor_copy(CSTK[:, s, 6:8], cloc_sb[:, s, :])
                selu_into(FNb[:, 2:256], psum_f[:, 0:254], bias1_sb[:], 254)
            else:
                C = GAT_DIMS[stage - 2][1]
                cnode = CSTK[:, s, 10 - 2 * stage:12 - 2 * stage]
                tcf = small.tile([128, 2], F32, tag="coord_t")
                nc.vector.tensor_scalar(tcf[:], cnode, cfac_sb[:, s, :], None,
                                        mybir.AluOpType.mult)
                cnw = small.tile([128, 2], F32, tag="cnw")
                nc.vector.tensor_add(cnw[:], dispagg, tcf[:])
                nc.vector.tensor_copy(FNb[:, 0:2], cnw[:])
                nc.vector.tensor_copy(CSTK[:, s, 8 - 2 * stage:10 - 2 * stage],
                                      cnw[:])
                nstk = 2 * (stage - 1)
                nc.vector.tensor_copy(FNb[:, 2:2 + nstk],
                                      CSTK[:, s, 10 - 2 * stage:8])
                selu_into(FNb[:, 2 + nstk:2 + nstk + C], psum_f[:, 0:C],
                          biasg_sb[stage - 1][:], C)
            if DBG and stage == 2 and s == 0:
                dfn = e_pool.tile([128, 512], F32, tag="dfn", bufs=1)
                nc.vector.tensor_copy(dfn[:], FNb[:])
                nc.sync.dma_start(dbg_fn[:], dfn[:])
            if DBG and stage == 3 and s == 0:
                dfn3 = e_pool.tile([128, 256], F32, tag="dfn3", bufs=1)
                nc.vector.tensor_copy(dfn3[:], FNb[:])
                nc.sync.dma_start(dbg_fn3[:], dfn3[:])
            if DBG and stage == 4 and s == 0:
                dfn4 = e_pool.tile([128, 128], F32, tag="dfn4", bufs=1)
                nc.vector.tensor_copy(dfn4[:], FNb[:])
                nc.sync.dma_start(dbg_fn4[:], dfn4[:])
            # shipped copy + bf16 sidecar
            nc.scalar.copy(FN8[:, 0:din_out], FNb[:])
            FN8b = FN8[:].bitcast(BF16) if FP8_ROWS else FN8[:]
            so = din_out // 2 if FP8_ROWS else din_out
            nc.scalar.copy(FN8b[:, so + 6:so + 8], FNb[:, 0:2])
            # scores for GAT layer `stage`
            ktf = din_out // 128
            psum_s = ps_sm.tile([128, 12], F32, tag="pssmall")
            for kt in range(ktf):
                pt = ps_sm.tile([128, 128], BF16, tag="pssmall")
                nc.tensor.transpose(pt[:], FNb[:, 128 * kt:128 * (kt + 1)], ident_sb[:])
                fnt = fnt_pool.tile([128, 128], BF16, tag="fnt")
                nc.scalar.copy(fnt[:], pt[:])
                nc.tensor.matmul(psum_s[:], fnt[:],
                                 wsc_sb[stage][:, 12 * kt:12 * (kt + 1)],
                                 start=(kt == 0), stop=(kt == ktf - 1))
            nc.vector.tensor_copy(FN8b[:, so:so + 6], psum_s[:, 0:6])
            nc.vector.tensor_copy(SDall[:, s, :], psum_s[:, 6:12])
            ship = FN8[:].bitcast(BF16) if FP8_ROWS else FN8[:]
            nc.sync.dma_start(
                agin[stage][s // 2][128 * (s % 2):128 * (s % 2 + 1), :], ship)

        # =========================================================
        def ag_chunk(stage, k):
            fl = NCORES * CHROWS * k
            nc.gpsimd.collective_compute(
                "AllGather", mybir.AluOpType.bypass, replica_groups=rg,
                ins=[agin[stage][k][:].opt()],
                outs=[feat[stage][fl:fl + NCORES * CHROWS, :].opt()])

        # chunk k holds slabs 2k, 2k+1. The trigger shares the in-order gpsimd
        # queue with the dma_gathers, which run ~3 slabs ahead of compute; a
        # trigger whose input (the slab 2k+1 produce DMA) is not yet written
        # when the queue reaches it BLOCKS all later gathers. Placing it just
        # before the gather of slab 2k+3 makes the queue arrive about when the
        # input lands: no meaningful blocking, minimal trigger delay.
        TRIG = {3: [0], 5: [1], 7: [2], 9: [3]}

        # STAGE 1: feat1 from data
        for s in range(NSLAB):
            for k in TRIG.get(s, []):
                ag_chunk(1, k)
            psum_f = ps_f.tile([128, 254], F32, tag="psum_f")
            nc.tensor.matmul(psum_f[:], dataT_sb[0:10, 128 * s:128 * (s + 1)],
                             linW_sb[0:10, :], start=True, stop=True)
            produce(1, s, psum_f, None)
        ag_chunk(1, 4)

        # =========================================================
        # STAGES 2..5: GAT layers 1..4
        for stage in range(2, 6):
            g = stage - 1
            din, C = GAT_DIMS[g - 1]
            kt = din // 128
            rb = ROWB[g - 1]
            so = din // 2 if FP8_ROWS else din
            ftab = feat[g]

            if g < 4:
                wp_t = wp_pool.tile([128, kt * 6 * C], BF16, tag="wp")
                nc.sync.dma_start(wp_t[:], inp[f"wp{g}"][:])

            for s in range(NSLAB):
                if stage < 5:
                    for k in TRIG.get(s, []):
                        ag_chunk(stage, k)
                # ---- gather src rows (bf16-typed transport) ----
                Fg = fg_pool.tile([128, 8, rb], BF16, tag="Fg")
                nc.gpsimd.dma_gather(Fg[:], ftab[:],
                                     srcidx_sb[:, 64 * s:64 * (s + 1)],
                                     1024, 1024, rb)
                Fgb = Fg[:]
                Fg8 = Fg[:].bitcast(F8) if FP8_ROWS else Fg[:]

                Ssl = S_sb[:, 128 * s:128 * (s + 1)]
                Stsl = St_sb[:, 128 * s:128 * (s + 1)]

                # ---- edge phase: segment softmax over incoming edges ----
                # dst-score broadcast: pbc[e,(b,h)] = SD[dst(b,e),h]
                Bsd = e_pool.tile([128, 8, 6], BF16, tag="Bsd")
                nc.vector.tensor_mul(
                    Bsd[:], mask_sb[:],
                    SDall[:, s, :].unsqueeze(1).broadcast_to([128, 8, 6]))
                pbc = ps_sm.tile([128, 8, 6], F32, tag="pbc")
                nc.tensor.matmul(pbc[:], Ssl, Bsd[:], start=True, stop=True)

                E_sl = e_pool.tile([128, 8, 6], F32, tag="E_sl")
                nc.vector.tensor_add(E_sl[:], Fgb[:, :, so:so + 6], pbc[:])
                E2 = e_pool.tile([128, 8, 6], F32, tag="E2")
                nc.scalar.activation(E2[:], E_sl[:], AF.Lrelu, alpha=0.2)
                EX = e_pool.tile([128, 8, 6], BF16, tag="EX")
                nc.scalar.activation(EX[:], E2[:], AF.Exp)

                # denominator reduce: D[L,(b,h)] += EX over edges of block b
                pdn = ps_sm.tile([128, 8, 6], F32, tag="pbc")
                nc.tensor.matmul(pdn[:], Stsl, EX[:], start=True, stop=True)
                Dm = e_pool.tile([128, 8, 6], F32, tag="Dm")
                nc.vector.tensor_mul(Dm[:], pdn[:], mask_sb[:])
                D4 = e_pool.tile([128, 4, 6], F32, tag="D4")
                nc.vector.tensor_add(D4[:], Dm[:, 0:4, :], Dm[:, 4:8, :])
                D2 = e_pool.tile([128, 2, 6], F32, tag="D2")
                nc.vector.tensor_add(D2[:], D4[:, 0:2, :], D4[:, 2:4, :])
                dple = e_pool.tile([128, 6], F32, tag="dple")
                nc.vector.tensor_add(dple[:], D2[:, 0, :], D2[:, 1, :])
                dpe = e_pool.tile([128, 6], F32, tag="dpe")
                nc.vector.tensor_scalar_add(dpe[:], dple[:], 1e-16)
                rd = e_pool.tile([128, 6], F32, tag="rd")
                nc.vector.reciprocal(rd[:], dpe[:])

                # reciprocal broadcast back to edges
                Brd = e_pool.tile([128, 8, 6], BF16, tag="Brd")
                nc.vector.tensor_mul(
                    Brd[:], mask_sb[:],
                    rd[:].unsqueeze(1).broadcast_to([128, 8, 6]))
                prd = ps_sm.tile([128, 8, 6], F32, tag="pbc")
                nc.tensor.matmul(prd[:], Ssl, Brd[:], start=True, stop=True)
                A_sl = e_pool.tile([128, 8, 6], BF16, tag="A_sl")
                nc.vector.tensor_mul(A_sl[:], EX[:], prd[:])

                # ---- attention-weighted src coordinate aggregation ----
                # amv = sum_h alpha (x6 mean folded into maskd)
                a3 = e_pool.tile([128, 8, 3], F32, tag="a3")
                nc.vector.tensor_add(a3[:], A_sl[:, :, 0:3], A_sl[:, :, 3:6])
                a2 = e_pool.tile([128, 8, 1], F32, tag="a2")
                nc.vector.tensor_add(a2[:], a3[:, :, 0:1], a3[:, :, 1:2])
                amv = e_pool.tile([128, 8, 1], F32, tag="amv")
                nc.vector.tensor_add(amv[:], a2[:], a3[:, :, 2:3])
                Bd = e_pool.tile([128, 8, 2], BF16, tag="Bd")
                nc.vector.tensor_mul(Bd[:], Fgb[:, :, so + 6:so + 8],
                                     amv[:].broadcast_to([128, 8, 2]))
                pdisp = ps_sm.tile([128, 8, 2], F32, tag="pbc")
                nc.tensor.matmul(pdisp[:], Stsl, Bd[:], start=True, stop=True)
                Dd = e_pool.tile([128, 8, 2], F32, tag="Dd")
                nc.vector.tensor_mul(Dd[:], pdisp[:], maskd_sb[:])
                Dd4 = e_pool.tile([128, 4, 2], F32, tag="Dd4")
                nc.vector.tensor_add(Dd4[:], Dd[:, 0:4, :], Dd[:, 4:8, :])
                Dd2 = e_pool.tile([128, 2, 2], F32, tag="Dd2")
                nc.vector.tensor_add(Dd2[:], Dd4[:, 0:2, :], Dd4[:, 2:4, :])
                dispagg = e_pool.tile([128, 2], F32, tag="dispagg")
                nc.vector.tensor_add(dispagg[:], Dd2[:, 0, :], Dd2[:, 1, :])

                if DBG and stage == 2 and s == 0:
                    dfg = fg_pool.tile([128, 8 * ROWB[0]], F32, tag="dbgfg", bufs=1)
                    nc.vector.tensor_copy(dfg[:], Fg[:].rearrange("p b e -> p (b e)"))
                    nc.sync.dma_start(dbg_fg[:], dfg[:])
                    dex = e_pool.tile([128, 8, 6], F32, tag="dex")
                    nc.vector.tensor_copy(dex[:], EX[:])
                    nc.sync.dma_start(dbg_ex[:], dex[:].rearrange("p b h -> p (b h)"))
                    dal = e_pool.tile([128, 8, 6], F32, tag="dal")
                    nc.vector.tensor_copy(dal[:], A_sl[:])
                    nc.sync.dma_start(dbg_al[:], dal[:].rearrange("p b h -> p (b h)"))
                    nc.sync.dma_start(dbg_da[:], dispagg[:])
                if stage == 5:
                    cnode = CSTK[:, s, 2:4]
                    tcf = small.tile([128, 2], F32, tag="coord_t")
                    nc.vector.tensor_scalar(tcf[:], cnode, cfac_sb[:, s, :], None,
                                            mybir.AluOpType.mult)
                    nc.vector.tensor_add(OUTC[:, s, :], dispagg[:], tcf[:])
                    continue

                # ---- feature aggregation ----
                M_sl = m_pool.tile([128, 8, 96], BF16, tag="M_sl")
                nc.vector.tensor_mul(
                    M_sl[:].rearrange("p b (h d) -> p b h d", h=6),
                    p0rep_sb[:, 96 * 8 * s:96 * 8 * (s + 1)]
                    .rearrange("p (b h d) -> p b h d", b=8, h=6),
                    A_sl[:].unsqueeze(3).broadcast_to([128, 8, 6, 16]))

                # scatter: Gt cols [ds][h*128 + b*16 + dl]
                Gt = gt_pool.tile([128, kt, 768], BF16, tag="Gt")
                for b in range(8):
                    pgt = ps_gt.tile([128, kt * 96], F32, tag="pgt")
                    for ds in range(kt):
                        nc.tensor.matmul(pgt[:, 96 * ds:96 * (ds + 1)],
                                         Fg8[:, b, 128 * ds:128 * (ds + 1)],
                                         M_sl[:, b, :], start=True, stop=True)
                    eng_copy = (nc.vector.tensor_copy if b % 4 == 0
                                else nc.scalar.copy)
                    eng_copy(
                        Gt[:].rearrange("p d (h2 b2 e) -> p d h2 b2 e",
                                        h2=6, b2=8)[:, :, :, b, :],
                        pgt[:].rearrange("p (d h2 e) -> p d h2 e", d=kt, h2=6))

                # feature matmul
                psum_f = ps_f.tile([128, C], F32, tag="psum_f")
                nmm = kt * 6
                i_mm = 0
                for ds in range(kt):
                    for h in range(6):
                        nc.tensor.matmul(psum_f[:], Gt[:, ds, 128 * h:128 * (h + 1)],
                                         wp_t[:, (ds * 6 + h) * C:(ds * 6 + h + 1) * C],
                                         start=(i_mm == 0), stop=(i_mm == nmm - 1))
                        i_mm += 1

                if DBG and stage == 2 and s == 0:
                    dpf = e_pool.tile([128, 508], F32, tag="dpf", bufs=1)
                    nc.vector.tensor_copy(dpf[:], psum_f[:, 0:C])
                    nc.sync.dma_start(dbg_pf[:, 0:C], dpf[:, 0:C])
                produce(stage, s, psum_f, dispagg[:])
                if s == NSLAB - 1:
                    ag_chunk(stage, 4)

        nc.sync.dma_start(out_t[:].rearrange("(s p) c -> p s c", p=128), OUTC[:])

    nc.compile()
    return nc


# ================================================================
def _host_prep(inputs):
    data = np.asarray(inputs["data"], np.float32)
    eidx = np.asarray(inputs["edge_idx"])
    src_a, dst_a = eidx[0].astype(np.int64), eidx[1].astype(np.int64)
    order = np.argsort(dst_a, kind="stable")
    src_s, dst_s = src_a[order], dst_a[order]
    indeg = np.bincount(dst_a, minlength=N)

    shared = {}
    linW = np.zeros((16, 254), np.float32)
    linW[0:10] = np.asarray(inputs["lin_W"], np.float32)
    shared["linW"] = linW
    shared["bias1"] = np.tile(np.asarray(inputs["lin_b"], np.float32)[None, :], (128, 1))
    shared["ident"] = np.eye(128, dtype=BF)
    maskrep = np.zeros((128, 8, 6), np.float32)
    for L in range(128):
        maskrep[L, L // 16, :] = 1.0
    shared["maskrep"] = maskrep.reshape(128, 48)
    shared["maskd"] = (maskrep[:, :, 0:2] / 6.0).reshape(128, 16).copy()
    for i in range(1, 5):
        din, C = GAT_DIMS[i - 1]
        kt = din // 128
        W = np.asarray(inputs[f"W{i}"], np.float32).reshape(din, H, C)
        if i < 4:
            wp = W / H
            shared[f"biasg{i}"] = np.tile(np.asarray(inputs[f"b{i}"], np.float32)[None, :], (128, 1))
            wp_h = np.zeros((128, kt * H * C), np.float32)
            for ds in range(kt):
                wp_h[:, ds * H * C:(ds + 1) * H * C] = \
                    wp[ds * 128:(ds + 1) * 128].reshape(128, H * C)
            shared[f"wp{i}"] = wp_h.astype(BF)
        a_s = np.asarray(inputs[f"as{i}"], np.float32)
        a_d = np.asarray(inputs[f"ad{i}"], np.float32)
        ws = np.einsum("dhc,hc->dh", W, a_s)
        wd = np.einsum("dhc,hc->dh", W, a_d)
        wsc = np.concatenate([ws, wd], 1)
        ktf = FDIM[i - 1] // 128
        wsc_h = np.zeros((128, ktf * 12), np.float32)
        for ds in range(ktf):
            wsc_h[:, ds * 12:(ds + 1) * 12] = wsc[ds * 128:(ds + 1) * 128]
        shared[f"wsc{i}"] = wsc_h.astype(BF)

    in_maps = []
    for r in range(NCORES):
        m = dict(shared)
        lo, hi = NL * r, NL * (r + 1)
        dT = np.zeros((16, NLP), np.float32)
        dT[0:10, 0:NL] = data[lo:hi].T
        m["dataT"] = dT
        cl = np.zeros((NLP, 2), np.float32)
        cl[0:NL] = data[lo:hi, 0:2]
        m["coords_loc"] = cl
        cf = np.ones((NLP, 1), np.float32)
        cf[0:NL, 0] = (indeg[lo:hi] == 0).astype(np.float32)
        m["cfac"] = cf

        sel = (dst_s >= lo) & (dst_s < hi)
        es, ed = src_s[sel], dst_s[sel] - lo
        p0rep = np.zeros((128, NBLK * 96), np.float32)
        Sbc = np.zeros((128, NSLAB * 128), np.float32)
        Str = np.zeros((128, NSLAB * 128), np.float32)
        sidx = np.zeros((128, NBLK * 8), np.int16)
        blk = ed // 16
        for c in range(NBLK):
            emask = blk == c
            k = int(emask.sum())
            assert k <= CAP, f"block overflow core {r} blk {c}: {k}"
            if k == 0:
                continue
            srcs = es[emask]
            lds = ed[emask].astype(np.int64)
            dls = lds % 16
            p0c = np.zeros((128, 16), np.float32)
            p0c[np.arange(k), dls] = 1.0
            p0rep[:, 96 * c:96 * (c + 1)] = np.tile(p0c, (1, 6))
            s_i, b_i = c // 8, c % 8
            # slab-wide one-hot: S[16*b+dl, e] marks edge slot e of block b
            Sbc[16 * b_i + dls, 128 * s_i + np.arange(k)] = 1.0
            Str[np.arange(k), 128 * s_i + 16 * b_i + dls] = 1.0
            # gather row ids in the chunked feat table layout
            rr = srcs // NL
            ii = srcs % NL
            c_i = ii // CHROWS
            agrow = NCORES * CHROWS * c_i + CHROWS * rr + (ii % CHROWS)
            fulls = np.zeros(128, np.int64)
            fulls[:k] = agrow
            ws_ = sidx[:, 64 * s_i:64 * (s_i + 1)]
            for e_i in range(128):
                gk = 128 * b_i + e_i
                ws_[gk % 16, gk // 16] = fulls[e_i]
        for s_i in range(NSLAB):
            w = sidx[:, 64 * s_i:64 * (s_i + 1)]
            w[16:] = np.tile(w[:16], (7, 1))
        m["p0rep"] = p0rep.astype(BF)
        m["Sbc"] = Sbc.astype(BF)
        m["Str"] = Str.astype(BF)
        m["srcidx"] = sidx
        in_maps.append(m)
    return in_maps


_NC_CACHE = None


def kernel(**inputs):
    global _NC_CACHE
    in_maps = _host_prep(inputs)
    if _NC_CACHE is None:
        _NC_CACHE = _build_nc()
    res = run_bass_kernel_spmd(_NC_CACHE, in_maps, core_ids=list(range(NCORES)))
    out = np.zeros((N, 2), np.float32)
    for r in range(NCORES):
        out[NL * r:NL * (r + 1)] = res.results[r]["out"][:NL]
    return out


# revision 27
# speedup vs baseline: 1.2415x; 1.2415x over previous
"""DeformGAT (4-layer) Trainium2 kernel — 8 NeuronCores SPMD.

Sharding: nodes in 8 contiguous blocks of 1250 (padded to 1280); edges are
assigned to their dst node's core (edges pre-sorted by dst on host). Weights
replicated. Feature tables are shipped as fp8 rows with a bf16 sidecar
([din fp8 | 6 scores bf16 | 2 coords bf16 | pad]), roughly halving AllGather
and gather traffic; fp8 is only used on the attention/feature W-path (final
rel err ~1e-3, validated against the fp32 reference in numpy). Per layer each
core gathers src rows of the replicated table (dma_gather), computes the
per-edge segment softmax with three slab-wide one-hot matmuls (dst-score
broadcast, denominator reduce, reciprocal broadcast) plus vector masking,
aggregates features with fp8 scatter matmuls, aggregates the attention-
weighted src coordinates with a fourth one-hot matmul (bf16 sidecar), then
ships its produced rows via fine-grained chunked AllGathers that overlap the
slab loop.
"""
import numpy as np
import ml_dtypes
from contextlib import ExitStack

import concourse.bacc as bacc
import concourse.bass as bass
import concourse.tile as tile
import concourse.mybir as mybir
from concourse import library_config
from concourse.bass_utils import run_bass_kernel_spmd

F32 = mybir.dt.float32
BF16 = mybir.dt.bfloat16
F8 = mybir.dt.float8e4
I16 = mybir.dt.int16
AF = mybir.ActivationFunctionType
BF = ml_dtypes.bfloat16

NCORES = 8
N = 10000
E = 60000
H = 6
NL = 1250          # real nodes per core
NLP = 1280         # padded nodes per core
NBLK = 80          # dst blocks of 16 per core
NSLAB = 10         # slabs of 128 dst nodes (8 blocks)
CAP = 128          # edge capacity per block (= chunk)
NCH = 5            # AllGather chunks per stage (2 slabs each)
CHROWS = 256       # rows per AllGather chunk

# GAT layer dims (din, C). Stage s (2..5) runs GAT s-1.
GAT_DIMS = [(256, 508), (512, 250), (256, 120), (128, 20)]
FDIM = [256, 512, 256, 128]          # feat_s dim produced by stage s
FP8_ROWS = True
# ROWB: table row width in bf16 elements. All DRAM transport (agin, feat,
# AllGather, dma_gather) is bf16-typed: shipping fp8-typed tensors corrupts
# bf16 sidecar bytes whose low byte matches an fp8 NaN pattern (canonicalized
# to 0x7f in the datapath). Valid e4m3 feature bytes are <= 0xf7, so fp8
# byte-pairs can never alias a bf16 NaN/Inf — bf16 transport is bit-safe.
if FP8_ROWS:
    ROWB = [256, 384, 256, 128]      # din/2 fp8-byte-pairs + 8 sidecar elems
else:
    ROWB = [384, 640, 384, 256]      # din + 8 sidecar elems

SELU_L = 1.0507009873554805
SELU_A = 1.6732632423543772


def _build_nc():
    nc = bacc.Bacc("TRN2", target_bir_lowering=False, debug=False,
                   num_devices=NCORES)
    # ---------------- inputs ----------------
    inp = {}
    inp["dataT"] = nc.dram_tensor("dataT", [16, NLP], F32, kind="ExternalInput")
    inp["coords_loc"] = nc.dram_tensor("coords_loc", [NLP, 2], F32, kind="ExternalInput")
    inp["cfac"] = nc.dram_tensor("cfac", [NLP, 1], F32, kind="ExternalInput")
    inp["srcidx"] = nc.dram_tensor("srcidx", [128, NBLK * 8], I16, kind="ExternalInput")
    inp["Sbc"] = nc.dram_tensor("Sbc", [128, NSLAB * 128], BF16, kind="ExternalInput")
    inp["Str"] = nc.dram_tensor("Str", [128, NSLAB * 128], BF16, kind="ExternalInput")
    inp["maskrep"] = nc.dram_tensor("maskrep", [128, 8 * 6], F32, kind="ExternalInput")
    inp["maskd"] = nc.dram_tensor("maskd", [128, 8 * 2], F32, kind="ExternalInput")
    inp["p0rep"] = nc.dram_tensor("p0rep", [128, NBLK * 96], BF16, kind="ExternalInput")
    inp["ident"] = nc.dram_tensor("ident", [128, 128], BF16, kind="ExternalInput")
    inp["linW"] = nc.dram_tensor("linW", [16, 254], F32, kind="ExternalInput")
    inp["bias1"] = nc.dram_tensor("bias1", [128, 254], F32, kind="ExternalInput")
    for i in range(1, 5):
        din, C = GAT_DIMS[i - 1]
        kt = din // 128
        if i < 4:
            inp[f"wp{i}"] = nc.dram_tensor(f"wp{i}", [128, kt * 6 * C], BF16, kind="ExternalInput")
            inp[f"biasg{i}"] = nc.dram_tensor(f"biasg{i}", [128, C], F32, kind="ExternalInput")
        ktf = FDIM[i - 1] // 128
        inp[f"wsc{i}"] = nc.dram_tensor(f"wsc{i}", [128, ktf * 12], BF16, kind="ExternalInput")
    out_t = nc.dram_tensor("out", [NLP, 2], F32, kind="ExternalOutput")
    DBG = False
    if DBG:
        dbg_fg = nc.dram_tensor("dbg_fg", [128, 8 * ROWB[0]], F32, kind="ExternalOutput")
        dbg_ex = nc.dram_tensor("dbg_ex", [128, 48], F32, kind="ExternalOutput")
        dbg_al = nc.dram_tensor("dbg_al", [128, 48], F32, kind="ExternalOutput")
        dbg_pf = nc.dram_tensor("dbg_pf", [128, 508], F32, kind="ExternalOutput")
        dbg_da = nc.dram_tensor("dbg_da", [128, 2], F32, kind="ExternalOutput")
        dbg_fn = nc.dram_tensor("dbg_fn", [128, 512], F32, kind="ExternalOutput")
        dbg_fn3 = nc.dram_tensor("dbg_fn3", [128, 256], F32, kind="ExternalOutput")
        dbg_fn4 = nc.dram_tensor("dbg_fn4", [128, 128], F32, kind="ExternalOutput")

    rg = [list(range(NCORES))]

    with tile.TileContext(nc) as tc, ExitStack() as ctx:
        persist = ctx.enter_context(tc.tile_pool(name="persist", bufs=1))
        dram = ctx.enter_context(tc.tile_pool(name="dram", bufs=1, space="DRAM"))
        fg_pool = ctx.enter_context(tc.tile_pool(name="fg", bufs=4))
        gt_pool = ctx.enter_context(tc.tile_pool(name="gt", bufs=2))
        m_pool = ctx.enter_context(tc.tile_pool(name="m", bufs=2))
        e_pool = ctx.enter_context(tc.tile_pool(name="ep", bufs=3))
        fn_pool = ctx.enter_context(tc.tile_pool(name="fn", bufs=2))
        fn8_pool = ctx.enter_context(tc.tile_pool(name="fn8", bufs=2))
        fnt_pool = ctx.enter_context(tc.tile_pool(name="fnt", bufs=2))
        wp_pool = ctx.enter_context(tc.tile_pool(name="wp", bufs=2))
        small = ctx.enter_context(tc.tile_pool(name="small", bufs=3))
        ps_gt = ctx.enter_context(tc.tile_pool(name="psgt", bufs=2, space="PSUM"))
        ps_f = ctx.enter_context(tc.tile_pool(name="psf", bufs=2, space="PSUM"))
        ps_sm = ctx.enter_context(tc.tile_pool(name="pssm", bufs=2, space="PSUM"))

        nc.gpsimd.load_library(library_config.mlp)

        # ------------- resident loads -------------
        srcidx_sb = persist.tile([128, NBLK * 8], I16)
        nc.sync.dma_start(srcidx_sb[:], inp["srcidx"][:])
        S_sb = persist.tile([128, NSLAB * 128], BF16)
        nc.sync.dma_start(S_sb[:], inp["Sbc"][:])
        St_sb = persist.tile([128, NSLAB * 128], BF16)
        nc.sync.dma_start(St_sb[:], inp["Str"][:])
        mask_sb = persist.tile([128, 8, 6], F32)
        nc.sync.dma_start(mask_sb[:], inp["maskrep"][:].rearrange("p (b h) -> p b h", b=8))
        maskd_sb = persist.tile([128, 8, 2], F32)
        nc.sync.dma_start(maskd_sb[:], inp["maskd"][:].rearrange("p (b c) -> p b c", b=8))
        p0rep_sb = persist.tile([128, NBLK * 96], BF16)
        nc.sync.dma_start(p0rep_sb[:], inp["p0rep"][:])
        ident_sb = persist.tile([128, 128], BF16)
        nc.sync.dma_start(ident_sb[:], inp["ident"][:])
        dataT_sb = persist.tile([16, NLP], F32)
        nc.sync.dma_start(dataT_sb[:], inp["dataT"][:])
        linW_sb = persist.tile([16, 254], F32)
        nc.sync.dma_start(linW_sb[:], inp["linW"][:])
        bias1_sb = persist.tile([128, 254], F32)
        nc.sync.dma_start(bias1_sb[:], inp["bias1"][:])
        cloc_sb = persist.tile([128, NSLAB, 2], F32)
        nc.sync.dma_start(cloc_sb[:],
                          inp["coords_loc"][:].rearrange("(s p) c -> p s c", p=128))
        cfac_sb = persist.tile([128, NSLAB, 1], F32)
        nc.sync.dma_start(cfac_sb[:],
                          inp["cfac"][:].rearrange("(s p) c -> p s c", p=128))
        wsc_sb = {}
        for i in range(1, 5):
            ktf = FDIM[i - 1] // 128
            t = persist.tile([128, ktf * 12], BF16, tag=f"wsc{i}", name=f"wsc{i}_sb")
            nc.sync.dma_start(t[:], inp[f"wsc{i}"][:])
            wsc_sb[i] = t
        biasg_sb = {}
        for i in range(1, 4):
            C = GAT_DIMS[i - 1][1]
            t = persist.tile([128, C], F32, tag=f"biasg{i}", name=f"biasg{i}_sb")
            nc.sync.dma_start(t[:], inp[f"biasg{i}"][:])
            biasg_sb[i] = t

        # per-stage state
        SDall = persist.tile([128, NSLAB, 6], F32)
        CSTK = persist.tile([128, NSLAB, 8], F32)
        OUTC = persist.tile([128, NSLAB, 2], F32)

        # DRAM tables (fp8 rows). agin is split per AllGather chunk so each
        # collective depends only on the two slab writes that feed it.
        agin = {}
        feat = {}
        for s in range(1, 5):
            agin[s] = [dram.tile([CHROWS, ROWB[s - 1]], BF16,
                                 tag=f"agin{s}_{k}", name=f"agin{s}_{k}")
                       for k in range(NCH)]
            feat[s] = dram.tile([NCORES * NLP, ROWB[s - 1]], BF16, tag=f"feat{s}",
                                name=f"feat{s}")

        # =========================================================
        def selu_into(dst_ap, psum_ap, bias_ap, C):
            """dst = selu(psum[:, :C] + bias)  (dst may be bf16)"""
            t1 = e_pool.tile([128, C], F32, tag="selu_t1")
            nc.vector.tensor_add(t1[:], psum_ap, bias_ap)
            mn = e_pool.tile([128, C], F32, tag="selu_mn")
            nc.scalar.activation(mn[:], t1[:], AF.Relu, scale=-1.0)
            ex = e_pool.tile([128, C], F32, tag="selu_ex")
            nc.scalar.activation(ex[:], mn[:], AF.Exp, scale=-1.0)
            em = e_pool.tile([128, C], F32, tag="selu_em")
            nc.vector.tensor_scalar(em[:], ex[:], SELU_L * SELU_A,
                                    -SELU_L * SELU_A,
                                    mybir.AluOpType.mult, mybir.AluOpType.add)
            rp = e_pool.tile([128, C], F32, tag="selu_rp")
            nc.scalar.activation(rp[:], t1[:], AF.Relu, scale=SELU_L)
            nc.vector.tensor_add(dst_ap, em[:], rp[:])

        def produce(stage, s, psum_f, dispagg):
            """assemble the row for GAT layer `stage` of slab s; scores; ship.

            FNb (bf16 [128, din]) feeds the score transposes; FN8 is the
            shipped fp8 row with the bf16 sidecar."""
            din_out = FDIM[stage - 1]
            rb = ROWB[stage - 1]
            FNb = fn_pool.tile([128, din_out], BF16, tag="FNb")
            FN8 = fn8_pool.tile([128, 2 * rb] if FP8_ROWS else [128, rb],
                                F8 if FP8_ROWS else BF16, tag="FN8")
            if stage == 1:
                nc.vector.tensor_copy(FNb[:, 0:2], cloc_sb[:, s, :])
                nc.vector.tensor_copy(CSTK[:, s, 6:8], cloc_sb[:, s, :])
                selu_into(FNb[:, 2:256], psum_f[:, 0:254], bias1_sb[:], 254)
            else:
                C = GAT_DIMS[stage - 2][1]
                cnode = CSTK[:, s, 10 - 2 * stage:12 - 2 * stage]
                tcf = small.tile([128, 2], F32, tag="coord_t")
                nc.vector.tensor_scalar(tcf[:], cnode, cfac_sb[:, s, :], None,
                                        mybir.AluOpType.mult)
                cnw = small.tile([128, 2], F32, tag="cnw")
                nc.vector.tensor_add(cnw[:], dispagg, tcf[:])
                nc.vector.tensor_copy(FNb[:, 0:2], cnw[:])
                nc.vector.tensor_copy(CSTK[:, s, 8 - 2 * stage:10 - 2 * stage],
                                      cnw[:])
                nstk = 2 * (stage - 1)
                nc.vector.tensor_copy(FNb[:, 2:2 + nstk],
                                      CSTK[:, s, 10 - 2 * stage:8])
                selu_into(FNb[:, 2 + nstk:2 + nstk + C], psum_f[:, 0:C],
                          biasg_sb[stage - 1][:], C)
            if DBG and stage == 2 and s == 0:
                dfn = e_pool.tile([128, 512], F32, tag="dfn", bufs=1)
                nc.vector.tensor_copy(dfn[:], FNb[:])
                nc.sync.dma_start(dbg_fn[:], dfn[:])
            if DBG and stage == 3 and s == 0:
                dfn3 = e_pool.tile([128, 256], F32, tag="dfn3", bufs=1)
                nc.vector.tensor_copy(dfn3[:], FNb[:])
                nc.sync.dma_start(dbg_fn3[:], dfn3[:])
            if DBG and stage == 4 and s == 0:
                dfn4 = e_pool.tile([128, 128], F32, tag="dfn4", bufs=1)
                nc.vector.tensor_copy(dfn4[:], FNb[:])
                nc.sync.dma_start(dbg_fn4[:], dfn4[:])
            # shipped copy + bf16 sidecar
            nc.scalar.copy(FN8[:, 0:din_out], FNb[:])
            FN8b = FN8[:].bitcast(BF16) if FP8_ROWS else FN8[:]
            so = din_out // 2 if FP8_ROWS else din_out
            nc.scalar.copy(FN8b[:, so + 6:so + 8], FNb[:, 0:2])
            # scores for GAT layer `stage`
            ktf = din_out // 128
            psum_s = ps_sm.tile([128, 12], F32, tag="pssmall")
            for kt in range(ktf):
                pt = ps_sm.tile([128, 128], BF16, tag="pssmall")
                nc.tensor.transpose(pt[:], FNb[:, 128 * kt:128 * (kt + 1)], ident_sb[:])
                fnt = fnt_pool.tile([128, 128], BF16, tag="fnt")
                (nc.vector.tensor_copy if kt % 2 == 0 else nc.scalar.copy)(
                    fnt[:], pt[:])
                nc.tensor.matmul(psum_s[:], fnt[:],
                                 wsc_sb[stage][:, 12 * kt:12 * (kt + 1)],
                                 start=(kt == 0), stop=(kt == ktf - 1))
            nc.vector.tensor_copy(FN8b[:, so:so + 6], psum_s[:, 0:6])
            nc.vector.tensor_copy(SDall[:, s, :], psum_s[:, 6:12])
            ship = FN8[:].bitcast(BF16) if FP8_ROWS else FN8[:]
            nc.sync.dma_start(
                agin[stage][s // 2][128 * (s % 2):128 * (s % 2 + 1), :], ship)

        # =========================================================
        def ag_chunk(stage, k):
            fl = NCORES * CHROWS * k
            nc.gpsimd.collective_compute(
                "AllGather", mybir.AluOpType.bypass, replica_groups=rg,
                ins=[agin[stage][k][:].opt()],
                outs=[feat[stage][fl:fl + NCORES * CHROWS, :].opt()])

        # chunk k holds slabs 2k, 2k+1. The trigger shares the in-order gpsimd
        # queue with the dma_gathers, which run ~3 slabs ahead of compute; a
        # trigger whose input (the slab 2k+1 produce DMA) is not yet written
        # when the queue reaches it BLOCKS all later gathers. Placing it just
        # before the gather of slab 2k+3 makes the queue arrive about when the
        # input lands: no meaningful blocking, minimal trigger delay.
        TRIG = {3: [0], 5: [1], 7: [2], 9: [3]}

        # STAGE 1: feat1 from data
        for s in range(NSLAB):
            for k in TRIG.get(s, []):
                ag_chunk(1, k)
            psum_f = ps_f.tile([128, 254], F32, tag="psum_f")
            nc.tensor.matmul(psum_f[:], dataT_sb[0:10, 128 * s:128 * (s + 1)],
                             linW_sb[0:10, :], start=True, stop=True)
            produce(1, s, psum_f, None)
        ag_chunk(1, 4)

        # =========================================================
        # STAGES 2..5: GAT layers 1..4
        for stage in range(2, 6):
            g = stage - 1
            din, C = GAT_DIMS[g - 1]
            kt = din // 128
            rb = ROWB[g - 1]
            so = din // 2 if FP8_ROWS else din
            ftab = feat[g]

            if g < 4:
                wp_t = wp_pool.tile([128, kt * 6 * C], BF16, tag="wp")
                nc.sync.dma_start(wp_t[:], inp[f"wp{g}"][:])

            for s in range(NSLAB):
                if stage < 5:
                    for k in TRIG.get(s, []):
                        ag_chunk(stage, k)
                # ---- gather src rows (bf16-typed transport) ----
                Fg = fg_pool.tile([128, 8, rb], BF16, tag="Fg")
                nc.gpsimd.dma_gather(Fg[:], ftab[:],
                                     srcidx_sb[:, 64 * s:64 * (s + 1)],
                                     1024, 1024, rb)
                Fgb = Fg[:]
                Fg8 = Fg[:].bitcast(F8) if FP8_ROWS else Fg[:]

                Ssl = S_sb[:, 128 * s:128 * (s + 1)]
                Stsl = St_sb[:, 128 * s:128 * (s + 1)]

                # ---- edge phase: segment softmax over incoming edges ----
                # dst-score broadcast: pbc[e,(b,h)] = SD[dst(b,e),h]
                Bsd = e_pool.tile([128, 8, 6], BF16, tag="Bsd")
                nc.vector.tensor_mul(
                    Bsd[:], mask_sb[:],
                    SDall[:, s, :].unsqueeze(1).broadcast_to([128, 8, 6]))
                pbc = ps_sm.tile([128, 8, 6], F32, tag="pbc")
                nc.tensor.matmul(pbc[:], Ssl, Bsd[:], start=True, stop=True)

                E_sl = e_pool.tile([128, 8, 6], F32, tag="E_sl")
                nc.vector.tensor_add(E_sl[:], Fgb[:, :, so:so + 6], pbc[:])
                rp1 = e_pool.tile([128, 8, 6], F32, tag="rp1")
                nc.scalar.activation(rp1[:], E_sl[:], AF.Relu)
                rn02 = e_pool.tile([128, 8, 6], F32, tag="rn02")
                nc.scalar.activation(rn02[:], E_sl[:], AF.Relu, scale=-0.2)
                E2 = e_pool.tile([128, 8, 6], F32, tag="E2")
                nc.vector.tensor_tensor(E2[:], rp1[:], rn02[:],
                                        mybir.AluOpType.subtract)
                EX = e_pool.tile([128, 8, 6], BF16, tag="EX")
                nc.scalar.activation(EX[:], E2[:], AF.Exp)

                # denominator reduce: D[L,(b,h)] += EX over edges of block b
                pdn = ps_sm.tile([128, 8, 6], F32, tag="pbc")
                nc.tensor.matmul(pdn[:], Stsl, EX[:], start=True, stop=True)
                Dm = e_pool.tile([128, 8, 6], F32, tag="Dm")
                nc.vector.tensor_mul(Dm[:], pdn[:], mask_sb[:])
                D4 = e_pool.tile([128, 4, 6], F32, tag="D4")
                nc.vector.tensor_add(D4[:], Dm[:, 0:4, :], Dm[:, 4:8, :])
                D2 = e_pool.tile([128, 2, 6], F32, tag="D2")
                nc.vector.tensor_add(D2[:], D4[:, 0:2, :], D4[:, 2:4, :])
                dple = e_pool.tile([128, 6], F32, tag="dple")
                nc.vector.tensor_add(dple[:], D2[:, 0, :], D2[:, 1, :])
                dpe = e_pool.tile([128, 6], F32, tag="dpe")
                nc.vector.tensor_scalar_add(dpe[:], dple[:], 1e-16)
                rd = e_pool.tile([128, 6], F32, tag="rd")
                nc.vector.reciprocal(rd[:], dpe[:])

                # reciprocal broadcast back to edges
                Brd = e_pool.tile([128, 8, 6], BF16, tag="Brd")
                nc.vector.tensor_mul(
                    Brd[:], mask_sb[:],
                    rd[:].unsqueeze(1).broadcast_to([128, 8, 6]))
                prd = ps_sm.tile([128, 8, 6], F32, tag="pbc")
                nc.tensor.matmul(prd[:], Ssl, Brd[:], start=True, stop=True)
                A_sl = e_pool.tile([128, 8, 6], BF16, tag="A_sl")
                nc.vector.tensor_mul(A_sl[:], EX[:], prd[:])

                # ---- attention-weighted src coordinate aggregation ----
                # amv = sum_h alpha (x6 mean folded into maskd)
                a3 = e_pool.tile([128, 8, 3], F32, tag="a3")
                nc.vector.tensor_add(a3[:], A_sl[:, :, 0:3], A_sl[:, :, 3:6])
                a2 = e_pool.tile([128, 8, 1], F32, tag="a2")
                nc.vector.tensor_add(a2[:], a3[:, :, 0:1], a3[:, :, 1:2])
                amv = e_pool.tile([128, 8, 1], F32, tag="amv")
                nc.vector.tensor_add(amv[:], a2[:], a3[:, :, 2:3])
                Bd = e_pool.tile([128, 8, 2], BF16, tag="Bd")
                nc.vector.tensor_mul(Bd[:], Fgb[:, :, so + 6:so + 8],
                                     amv[:].broadcast_to([128, 8, 2]))
                pdisp = ps_sm.tile([128, 8, 2], F32, tag="pbc")
                nc.tensor.matmul(pdisp[:], Stsl, Bd[:], start=True, stop=True)
                Dd = e_pool.tile([128, 8, 2], F32, tag="Dd")
                nc.vector.tensor_mul(Dd[:], pdisp[:], maskd_sb[:])
                Dd4 = e_pool.tile([128, 4, 2], F32, tag="Dd4")
                nc.vector.tensor_add(Dd4[:], Dd[:, 0:4, :], Dd[:, 4:8, :])
                Dd2 = e_pool.tile([128, 2, 2], F32, tag="Dd2")
                nc.vector.tensor_add(Dd2[:], Dd4[:, 0:2, :], Dd4[:, 2:4, :])
                dispagg = e_pool.tile([128, 2], F32, tag="dispagg")
                nc.vector.tensor_add(dispagg[:], Dd2[:, 0, :], Dd2[:, 1, :])

                if DBG and stage == 2 and s == 0:
                    dfg = fg_pool.tile([128, 8 * ROWB[0]], F32, tag="dbgfg", bufs=1)
                    nc.vector.tensor_copy(dfg[:], Fg[:].rearrange("p b e -> p (b e)"))
                    nc.sync.dma_start(dbg_fg[:], dfg[:])
                    dex = e_pool.tile([128, 8, 6], F32, tag="dex")
                    nc.vector.tensor_copy(dex[:], EX[:])
                    nc.sync.dma_start(dbg_ex[:], dex[:].rearrange("p b h -> p (b h)"))
                    dal = e_pool.tile([128, 8, 6], F32, tag="dal")
                    nc.vector.tensor_copy(dal[:], A_sl[:])
                    nc.sync.dma_start(dbg_al[:], dal[:].rearrange("p b h -> p (b h)"))
                    nc.sync.dma_start(dbg_da[:], dispagg[:])
                if stage == 5:
                    cnode = CSTK[:, s, 2:4]
                    tcf = small.tile([128, 2], F32, tag="coord_t")
                    nc.vector.tensor_scalar(tcf[:], cnode, cfac_sb[:, s, :], None,
                                            mybir.AluOpType.mult)
                    nc.vector.tensor_add(OUTC[:, s, :], dispagg[:], tcf[:])
                    continue

                # ---- feature aggregation ----
                M_sl = m_pool.tile([128, 8, 96], BF16, tag="M_sl")
                nc.vector.tensor_mul(
                    M_sl[:].rearrange("p b (h d) -> p b h d", h=6),
                    p0rep_sb[:, 96 * 8 * s:96 * 8 * (s + 1)]
                    .rearrange("p (b h d) -> p b h d", b=8, h=6),
                    A_sl[:].unsqueeze(3).broadcast_to([128, 8, 6, 16]))

                # scatter: Gt cols [ds][h*128 + b*16 + dl]
                Gt = gt_pool.tile([128, kt, 768], BF16, tag="Gt")
                for b in range(8):
                    pgt = ps_gt.tile([128, kt * 96], F32, tag="pgt")
                    for ds in range(kt):
                        nc.tensor.matmul(pgt[:, 96 * ds:96 * (ds + 1)],
                                         Fg8[:, b, 128 * ds:128 * (ds + 1)],
                                         M_sl[:, b, :], start=True, stop=True)
                    eng_copy = (nc.vector.tensor_copy if b % 2 == 0
                                else nc.scalar.copy)
                    eng_copy(
                        Gt[:].rearrange("p d (h2 b2 e) -> p d h2 b2 e",
                                        h2=6, b2=8)[:, :, :, b, :],
                        pgt[:].rearrange("p (d h2 e) -> p d h2 e", d=kt, h2=6))

                # feature matmul
                psum_f = ps_f.tile([128, C], F32, tag="psum_f")
                nmm = kt * 6
                i_mm = 0
                for ds in range(kt):
                    for h in range(6):
                        nc.tensor.matmul(psum_f[:], Gt[:, ds, 128 * h:128 * (h + 1)],
                                         wp_t[:, (ds * 6 + h) * C:(ds * 6 + h + 1) * C],
                                         start=(i_mm == 0), stop=(i_mm == nmm - 1))
                        i_mm += 1

                if DBG and stage == 2 and s == 0:
                    dpf = e_pool.tile([128, 508], F32, tag="dpf", bufs=1)
                    nc.vector.tensor_copy(dpf[:], psum_f[:, 0:C])
                    nc.sync.dma_start(dbg_pf[:, 0:C], dpf[:, 0:C])
                produce(stage, s, psum_f, dispagg[:])
                if s == NSLAB - 1:
                    ag_chunk(stage, 4)

        nc.sync.dma_start(out_t[:].rearrange("(s p) c -> p s c", p=128), OUTC[:])

    nc.compile()
    return nc


# ================================================================
def _host_prep(inputs):
    data = np.asarray(inputs["data"], np.float32)
    eidx = np.asarray(inputs["edge_idx"])
    src_a, dst_a = eidx[0].astype(np.int64), eidx[1].astype(np.int64)
    order = np.argsort(dst_a, kind="stable")
    src_s, dst_s = src_a[order], dst_a[order]
    indeg = np.bincount(dst_a, minlength=N)

    shared = {}
    linW = np.zeros((16, 254), np.float32)
    linW[0:10] = np.asarray(inputs["lin_W"], np.float32)
    shared["linW"] = linW
    shared["bias1"] = np.tile(np.asarray(inputs["lin_b"], np.float32)[None, :], (128, 1))
    shared["ident"] = np.eye(128, dtype=BF)
    maskrep = np.zeros((128, 8, 6), np.float32)
    for L in range(128):
        maskrep[L, L // 16, :] = 1.0
    shared["maskrep"] = maskrep.reshape(128, 48)
    shared["maskd"] = (maskrep[:, :, 0:2] / 6.0).reshape(128, 16).copy()
    for i in range(1, 5):
        din, C = GAT_DIMS[i - 1]
        kt = din // 128
        W = np.asarray(inputs[f"W{i}"], np.float32).reshape(din, H, C)
        if i < 4:
            wp = W / H
            shared[f"biasg{i}"] = np.tile(np.asarray(inputs[f"b{i}"], np.float32)[None, :], (128, 1))
            wp_h = np.zeros((128, kt * H * C), np.float32)
            for ds in range(kt):
                wp_h[:, ds * H * C:(ds + 1) * H * C] = \
                    wp[ds * 128:(ds + 1) * 128].reshape(128, H * C)
            shared[f"wp{i}"] = wp_h.astype(BF)
        a_s = np.asarray(inputs[f"as{i}"], np.float32)
        a_d = np.asarray(inputs[f"ad{i}"], np.float32)
        ws = np.einsum("dhc,hc->dh", W, a_s)
        wd = np.einsum("dhc,hc->dh", W, a_d)
        wsc = np.concatenate([ws, wd], 1)
        ktf = FDIM[i - 1] // 128
        wsc_h = np.zeros((128, ktf * 12), np.float32)
        for ds in range(ktf):
            wsc_h[:, ds * 12:(ds + 1) * 12] = wsc[ds * 128:(ds + 1) * 128]
        shared[f"wsc{i}"] = wsc_h.astype(BF)

    in_maps = []
    for r in range(NCORES):
        m = dict(shared)
        lo, hi = NL * r, NL * (r + 1)
        dT = np.zeros((16, NLP), np.float32)
        dT[0:10, 0:NL] = data[lo:hi].T
        m["dataT"] = dT
        cl = np.zeros((NLP, 2), np.float32)
        cl[0:NL] = data[lo:hi, 0:2]
        m["coords_loc"] = cl
        cf = np.ones((NLP, 1), np.float32)
        cf[0:NL, 0] = (indeg[lo:hi] == 0).astype(np.float32)
        m["cfac"] = cf

        sel = (dst_s >= lo) & (dst_s < hi)
        es, ed = src_s[sel], dst_s[sel] - lo
        p0rep = np.zeros((128, NBLK * 96), np.float32)
        Sbc = np.zeros((128, NSLAB * 128), np.float32)
        Str = np.zeros((128, NSLAB * 128), np.float32)
        sidx = np.zeros((128, NBLK * 8), np.int16)
        blk = ed // 16
        for c in range(NBLK):
            emask = blk == c
            k = int(emask.sum())
            assert k <= CAP, f"block overflow core {r} blk {c}: {k}"
            if k == 0:
                continue
            srcs = es[emask]
            lds = ed[emask].astype(np.int64)
            dls = lds % 16
            p0c = np.zeros((128, 16), np.float32)
            p0c[np.arange(k), dls] = 1.0
            p0rep[:, 96 * c:96 * (c + 1)] = np.tile(p0c, (1, 6))
            s_i, b_i = c // 8, c % 8
            # slab-wide one-hot: S[16*b+dl, e] marks edge slot e of block b
            Sbc[16 * b_i + dls, 128 * s_i + np.arange(k)] = 1.0
            Str[np.arange(k), 128 * s_i + 16 * b_i + dls] = 1.0
            # gather row ids in the chunked feat table layout
            rr = srcs // NL
            ii = srcs % NL
            c_i = ii // CHROWS
            agrow = NCORES * CHROWS * c_i + CHROWS * rr + (ii % CHROWS)
            fulls = np.zeros(128, np.int64)
            fulls[:k] = agrow
            ws_ = sidx[:, 64 * s_i:64 * (s_i + 1)]
            for e_i in range(128):
                gk = 128 * b_i + e_i
                ws_[gk % 16, gk // 16] = fulls[e_i]
        for s_i in range(NSLAB):
            w = sidx[:, 64 * s_i:64 * (s_i + 1)]
            w[16:] = np.tile(w[:16], (7, 1))
        m["p0rep"] = p0rep.astype(BF)
        m["Sbc"] = Sbc.astype(BF)
        m["Str"] = Str.astype(BF)
        m["srcidx"] = sidx
        in_maps.append(m)
    return in_maps


_NC_CACHE = None


def kernel(**inputs):
    global _NC_CACHE
    in_maps = _host_prep(inputs)
    if _NC_CACHE is None:
        _NC_CACHE = _build_nc()
    res = run_bass_kernel_spmd(_NC_CACHE, in_maps, core_ids=list(range(NCORES)))
    out = np.zeros((N, 2), np.float32)
    for r in range(NCORES):
        out[NL * r:NL * (r + 1)] = res.results[r]["out"][:NL]
    return out
